# revision 41
# baseline (speedup 1.0000x reference)
"""Multi-head attention (B=4, S=2048, D=1024, H=16, dk=dv=64) on 8 TRN2 cores.

Sharding: core c = 2*b + hg handles batch b = c//2 and heads
[hg*8, hg*8+8). Each core computes a partial output (its 8 heads'
contribution through Wo); the host adds the two partials per batch.

v2 design (vs the serial-matmul baseline): the PE array is addressed in
32-strip tiles so the half-array attention matmuls run CONCURRENTLY
(hw-measured 2.0x):

  - scores: K=dk=64 -> the two heads of a pair run as a row-tiled wave
    (h0 in array rows 0-63, h1 in rows 64-127), both N=512 matmuls
    retire in ~216ns total. Outputs land in one [128,1024] PSUM tile
    (h0 cols 0:512 | h1 cols 512:1024) so ONE ScalarE exp instruction
    covers the pair.
  - mix: M=dv=64 -> col-tiled wave: h0's mix into PSUM partitions 0:64
    (array cols 0-63), h1's into 64:128, accumulating over the 16 key
    chunks into a single 1-bank [128,512] tile that is ALREADY in the
    normalized-lhsT layout Wo wants.
  - softmax denominators: one 4-way col-tiled PE wave per key chunk
    (M=1 mask column per head at array cols 0/32/64/96, hw-measured
    ~217ns for all 4 heads) accumulating [1,512] rows into a single
    PSUM bank at partitions 0/32/64/96; DVE evacuates + reciprocal,
    GpSimd broadcasts across partitions (~0.9us), DVE multiplies the
    mix PSUM -> normalized bf16.

ScalarE runs only the 256 exp instructions (~280us) and is the
bottleneck engine; everything else is scheduled so exp never starves.
The loop is block-major (block = 2 pairs = 4 heads) so the k/v
projection build cost spreads over 4 query blocks of ACT time per
block: for blk in (pairs 01, pairs 23): for qb in 4: 16 key-chunk
iterations of [fillers | scores wave P | scores wave Q | exp P | exp Q
| mix waves (kc-1) | sums wave (kc-1)]; then normalize. Wo for query
block qb runs as fillers once blk1's normalize for qb lands
(accumulate over the 4 pairs, K=128).

Fillers (k/q/v projections, Wo) are deadline-ordered thunks consumed
at the top of each kc iteration; they accumulate in half-bank
[128,256] PSUM tiles (ping-pong in one bank) with evacs on DVE.

PSUM: sc ring 2x[128,1024] (4 banks), mix 2x[128,512] (2 banks),
sums 1x[128,512] (1 bank), aux 2x[128,256] (1 bank).
"""

import numpy as np

B, S, D = 4, 2048, 1024
H, DK, DV = 16, 64, 64
HC = 8          # heads per core
NP = HC // 2    # head pairs per core
NCORES = 8
NC_CHUNKS = D // 128    # 8 contraction chunks over D
NKC = S // 128          # 16 key chunks
NQB = S // 512          # 4 query blocks

_COMPILED = {}


def _build_nc():
    import concourse.tile as tile
    from concourse import bacc, mybir
    from contextlib import ExitStack

    F32 = mybir.dt.float32
    BF16 = mybir.dt.bfloat16
    EXP = mybir.ActivationFunctionType.Exp

    nc = bacc.Bacc("TRN2", target_bir_lowering=False, debug=False,
                   num_devices=NCORES)

    qT = nc.dram_tensor("qT", [D, S], BF16, kind="ExternalInput").ap()
    kT = nc.dram_tensor("kT", [D, S], BF16, kind="ExternalInput").ap()
    vT = nc.dram_tensor("vT", [D, S], BF16, kind="ExternalInput").ap()
    wq = nc.dram_tensor("wq", [D, HC * DK], BF16, kind="ExternalInput").ap()
    wk = nc.dram_tensor("wk", [D, HC * DK], BF16, kind="ExternalInput").ap()
    wv = nc.dram_tensor("wv", [D, HC * DV], BF16, kind="ExternalInput").ap()
    wo = nc.dram_tensor("wo", [HC * DV, D], BF16, kind="ExternalInput").ap()
    maskr = nc.dram_tensor("maskr", [128, NKC], F32, kind="ExternalInput").ap()
    # bf16 partial output (the host sums the two per-batch partials in
    # f32; bf16 quantization of the partials adds ~2e-3 rel err, well
    # inside budget, and halves the tail writeback DMA)
    out = nc.dram_tensor("out", [S, D], BF16, kind="ExternalOutput").ap()

    with tile.TileContext(nc) as tc:
        with ExitStack() as ctx:
            const_pool = ctx.enter_context(tc.tile_pool(name="const", bufs=1))
            w_pool = ctx.enter_context(tc.tile_pool(name="weights", bufs=1))
            act_pool = ctx.enter_context(tc.tile_pool(name="acts", bufs=1))
            kst_pool = ctx.enter_context(
                tc.tile_pool(name="kstage", bufs=4 * NC_CHUNKS))
            qst_pool = ctx.enter_context(
                tc.tile_pool(name="qstage", bufs=2 * NC_CHUNKS))
            vt_pool = ctx.enter_context(tc.tile_pool(name="vstage", bufs=2))
            exp_pool = ctx.enter_context(tc.tile_pool(name="exp", bufs=6))
            nt_pool = ctx.enter_context(tc.tile_pool(name="norm", bufs=12))
            sums_sb_pool = ctx.enter_context(
                tc.tile_pool(name="sumssb", bufs=4))
            recb_pool = ctx.enter_context(tc.tile_pool(name="recb", bufs=4))
            osb_pool = ctx.enter_context(tc.tile_pool(name="outsb", bufs=6))
            sc_pool = ctx.enter_context(
                tc.tile_pool(name="scpsum", bufs=2, space="PSUM"))
            mix_pool = ctx.enter_context(
                tc.tile_pool(name="mxpsum", bufs=2, space="PSUM"))
            sums_pool = ctx.enter_context(
                tc.tile_pool(name="smpsum", bufs=1, space="PSUM"))
            aux_pool = ctx.enter_context(
                tc.tile_pool(name="auxpsum", bufs=1, space="PSUM"))
            dram_pool = ctx.enter_context(
                tc.tile_pool(name="dscratch", bufs=4, space="DRAM"))

            mask_sb = const_pool.tile([128, NKC], F32)
            nc.sync.dma_start(mask_sb[:], maskr[:])
            mask_bf = const_pool.tile([128, NKC], BF16)
            nc.vector.tensor_copy(mask_bf[:], mask_sb[:])
            # PE warmup: dummy matmuls on resident SBUF data bridge the
            # initial weight/staging DMA wait at full HAM ramp.
            warm_sb = const_pool.tile([128, 512], BF16)
            nc.vector.memset(warm_sb[:], 0.0)
            warm_ps = sums_pool.tile([128, 512], F32, tag="sums",
                                     name="warmps")
            for i in range(26):
                nc.tensor.matmul(warm_ps[:], lhsT=warm_sb[:, 0:128],
                                 rhs=warm_sb[:],
                                 start=(i == 0), stop=(i == 25))
            warm_out = const_pool.tile([128, 512], F32)
            nc.vector.tensor_copy(warm_out[:], warm_ps[:])

            # DMA priority: wk + k staging first (head critical path),
            # then wq + q staging; wv before attention, wo much later.
            wq_sb = w_pool.tile([128, NC_CHUNKS * 512], BF16, tag="wq")
            wk_sb = w_pool.tile([128, NC_CHUNKS * 512], BF16, tag="wk")
            wv_sb = w_pool.tile([128, NC_CHUNKS * 512], BF16, tag="wv")
            wo_sb = w_pool.tile([128, NP * 1024], BF16, tag="wo")
            for c in range(NC_CHUNKS):
                nc.sync.dma_start(wk_sb[:, c * 512:(c + 1) * 512],
                                  wk[c * 128:(c + 1) * 128, :])

            # persistent activations
            qhTb = [[act_pool.tile([128, 512], BF16, tag=f"qhT{p}_{b}",
                                   name=f"qhT{p}_{b}") for b in range(NQB)]
                    for p in range(NP)]
            khT = [act_pool.tile([128, S], BF16, tag=f"khT{p}",
                                 name=f"khT{p}") for p in range(NP)]
            khTb = [[khT[p][:, b * 512:(b + 1) * 512] for b in range(NQB)]
                    for p in range(NP)]
            vhs_all = act_pool.tile([128, NKC * 512], BF16, tag="vhall")
            vhs = [vhs_all[:, t * 512:(t + 1) * 512] for t in range(NKC)]

            def stage_block(pool, src, blk):
                stg = []
                for c in range(NC_CHUNKS):
                    t = pool.tile([128, 512], BF16, tag="stage",
                                  name=f"stg{c}")
                    nc.sync.dma_start(
                        t[:],
                        src[c * 128:(c + 1) * 128,
                            blk * 512:(blk + 1) * 512])
                    stg.append(t)
                return stg

            # The single aux PSUM bank hosts all filler accumulation.
            # Each group splits its output into the two 256-col halves
            # of one shared tile (one start/stop pair per group - two
            # starts in one 2KB zero region is illegal). Region-level
            # dependency tracking then lets group i+1's first matmul
            # wait only on the evac of group i's first half, which
            # drains while group i's second half is still accumulating:
            # the groups pipeline instead of serializing on the bank.
            # Any OTHER aux allocation invalidates the shared tile.
            aux_state = {"n": 0, "tile": None, "tile_n": -1}

            def aux_alloc(name):
                aux_state["n"] += 1
                return aux_pool.tile([128, 512], F32, tag="aux", name=name)

            def aux_halves(name):
                if aux_state["tile_n"] != aux_state["n"]:
                    aux_state["tile"] = aux_alloc(name)
                    aux_state["tile_n"] = aux_state["n"]
                t = aux_state["tile"]
                return t[:, 0:256], t[:, 256:512]

            # q/k projection: 8 K-chunks x two 256-token halves
            # (sharing each lhsT load). memset-then-accumulate: the
            # memsets and evacs of adjacent groups overlap the other
            # half's matmuls, so groups pipeline through the one bank.
            def proj_group_run(kind, stg, wsb, dst_tile, p, tag):
                psA, psB = aux_halves(f"pj{tag}")
                nc.vector.memset(psA, 0.0)
                nc.vector.memset(psB, 0.0)
                for c in range(NC_CHUNKS):
                    lhsT = wsb[:, c * 512 + p * 128:c * 512 + (p + 1) * 128]
                    nc.tensor.matmul(psA, lhsT=lhsT, rhs=stg[c][:, 0:256],
                                     start=False, stop=False,
                                     skip_group_check=True)
                    nc.tensor.matmul(psB, lhsT=lhsT, rhs=stg[c][:, 256:512],
                                     start=False, stop=False,
                                     skip_group_check=True)
                nc.vector.tensor_copy(dst_tile[:, 0:256], psA)
                nc.vector.tensor_copy(dst_tile[:, 256:512], psB)

            # v projection: per key chunk t and head-half (4 heads =
            # 256 cols), 8 matmuls N=256 into one aux accumulator.
            vt_cur = {}

            # v projection: key chunk t x head-half; 8 matmuls N=256
            # into one half of the shared aux tile (alternating by t).
            def vproj_group_run(t, half):
                vt = vt_cur[t // 4]
                o = t % 4
                psA, psB = aux_halves(f"vpj{t}_{half}")
                ps = psA if t % 2 == 0 else psB
                nc.vector.memset(ps, 0.0)
                for c in range(NC_CHUNKS):
                    nc.tensor.matmul(
                        ps,
                        lhsT=vt[:, c * 512 + o * 128:
                                c * 512 + (o + 1) * 128],
                        rhs=wv_sb[:, c * 512 + half * 256:
                                  c * 512 + (half + 1) * 256],
                        start=False, stop=False,
                        skip_group_check=True)
                # mask the value rows (masked keys contribute 0)
                nc.vector.tensor_scalar_mul(
                    vhs[t][:, half * 256:(half + 1) * 256],
                    ps, mask_sb[:, t:t + 1])

            def stage_vq(vq):
                vt = vt_pool.tile([128, NC_CHUNKS * 512], BF16, tag="vt",
                                  name=f"vq{vq}")
                for c in range(NC_CHUNKS):
                    nc.sync.dma_start(
                        vt[:, c * 512:(c + 1) * 512],
                        vT[c * 128:(c + 1) * 128,
                           vq * 512:(vq + 1) * 512])
                vt_cur[vq] = vt

            # Wo: one group = (tt, dh): accumulate over the 4 pairs
            # into the two shared-aux halves (halves share each lhsT
            # load), evac (bf16) + DMA out at the end.
            def wo_group_run(qb, nts, tt, dh):
                psA, psB = aux_halves(f"wo{tt}{dh}")
                nc.vector.memset(psA, 0.0)
                nc.vector.memset(psB, 0.0)
                for p in range(NP):
                    lhsT = nts[p][:, tt * 128:(tt + 1) * 128]
                    rbase = p * 1024 + dh * 512
                    nc.tensor.matmul(psA, lhsT=lhsT,
                                     rhs=wo_sb[:, rbase:rbase + 256],
                                     start=False, stop=False,
                                     skip_group_check=True)
                    nc.tensor.matmul(psB, lhsT=lhsT,
                                     rhs=wo_sb[:, rbase + 256:rbase + 512],
                                     start=False, stop=False,
                                     skip_group_check=True)
                osb = osb_pool.tile([128, 512], BF16, tag="osb")
                nc.vector.tensor_copy(osb[:, 0:256], psA)
                nc.vector.tensor_copy(osb[:, 256:512], psB)
                nc.sync.dma_start(
                    out[qb * 512 + tt * 128:qb * 512 + (tt + 1) * 128,
                        dh * 512:(dh + 1) * 512], osb[:])

            # ---- attention waves ----
            def scores_wave(p, qb, kc, sc):
                kb, ko = kc // 4, kc % 4
                ksl = slice(ko * 128, (ko + 1) * 128)
                nc.tensor.matmul(sc[:, 0:512],
                                 lhsT=khTb[p][kb][0:64, ksl],
                                 rhs=qhTb[p][qb][0:64, :],
                                 start=True, stop=True)
                nc.tensor.matmul(sc[:, 512:1024],
                                 lhsT=khTb[p][kb][64:128, ksl],
                                 rhs=qhTb[p][qb][64:128, :],
                                 start=True, stop=True)

            # mix/sums accumulate onto DVE-memset zeros with
            # start=False on every matmul: a col-tiled bank hosts two
            # accumulation streams, and a start_tensor_calc by either
            # would lazily re-zero the whole 2KB region under the
            # other's partial sums. Accumulating onto true zeros is
            # correct under both has_written semantics.
            def mix_wave(p, kc, ex, mixps):
                h0 = vhs[kc][:, (2 * p % HC) * 64:(2 * p % HC) * 64 + 64]
                h1 = vhs[kc][:, ((2 * p + 1) % HC) * 64:
                             ((2 * p + 1) % HC) * 64 + 64]
                nc.tensor.matmul(mixps[0:64, :], lhsT=h0, rhs=ex[:, 0:512],
                                 start=False, stop=False,
                                 skip_group_check=True,
                                 tile_position=(0, 0))
                nc.tensor.matmul(mixps[64:128, :], lhsT=h1,
                                 rhs=ex[:, 512:1024],
                                 start=False, stop=False,
                                 skip_group_check=True,
                                 tile_position=(0, 64))

            def sums_wave(kc, exA, exB, sums):
                mcol = mask_bf[:, kc:kc + 1]
                for j, rhs in enumerate((exA[:, 0:512], exA[:, 512:1024],
                                         exB[:, 0:512], exB[:, 512:1024])):
                    nc.tensor.matmul(sums[32 * j:32 * j + 1, :],
                                     lhsT=mcol, rhs=rhs,
                                     start=False, stop=False,
                                     skip_group_check=True,
                                     tile_position=(0, 32 * j))

            def recip_sums(sums, tag):
                # one reciprocal over the sums bank (rows 1-31 etc are
                # memset zeros -> inf, never read), PSUM -> SBUF.
                rec = sums_sb_pool.tile([97, 512], F32, tag="ssb",
                                        name=f"rec{tag}")
                nc.vector.reciprocal_approx_fast(rec[:], sums[0:97, :])
                return rec

            def normalize(mixps, rec, j0, pair_tag):
                # broadcast each head's reciprocal row over its 64
                # partitions via a DRAM bounce (partition_broadcast is
                # wrong on hw), multiply the mix PSUM -> bf16.
                dsc = dram_pool.tile([2, 512], F32, tag="dsc")
                nc.sync.dma_start(dsc[0:1, :], rec[32 * j0:32 * j0 + 1, :])
                nc.sync.dma_start(dsc[1:2, :],
                                  rec[32 * j0 + 32:32 * j0 + 33, :])
                recb = recb_pool.tile([128, 512], F32, tag="recb")
                nc.sync.dma_start(recb[0:64, :],
                                  dsc[0:1, :].to_broadcast((64, 512)))
                nc.sync.dma_start(recb[64:128, :],
                                  dsc[1:2, :].to_broadcast((64, 512)))
                nt = nt_pool.tile([128, 512], BF16, tag="norm",
                                  name=f"nt{pair_tag}")
                nc.vector.tensor_mul(nt[:], mixps[:], recb[:])
                return nt

            # ---- startup: stage + project what the first exps need ----
            kstg = [None] * NQB
            qstg = {}
            # DMA order is the head critical path: wk+k0, wq+q0 feed the
            # first four projection groups; wv/v-quarters are only
            # needed by the v-projections a few kc in.
            kstg[0] = stage_block(kst_pool, kT, 0)
            for c in range(NC_CHUNKS):
                nc.sync.dma_start(wq_sb[:, c * 512:(c + 1) * 512],
                                  wq[c * 128:(c + 1) * 128, :])
            qstg[0] = stage_block(qst_pool, qT, 0)
            proj_group_run("k", kstg[0], wk_sb, khTb[0][0], 0, "k00")
            proj_group_run("q", qstg[0], wq_sb, qhTb[0][0], 0, "q00")
            proj_group_run("k", kstg[0], wk_sb, khTb[1][0], 1, "k10")
            proj_group_run("q", qstg[0], wq_sb, qhTb[1][0], 1, "q10")
            # DMA barrier: the sync queue races ahead of the PE, so a
            # dummy DMA depending on the last startup projection keeps
            # the wv/v-quarter (and later) transfers out of the DMA
            # rings until the head-critical wk/k0/wq/q0 bytes land.
            bar = dram_pool.tile([1, 16], BF16, tag="dsc", name="bar")
            nc.sync.dma_start(bar[:], qhTb[1][0][0:1, 0:16])
            for c in range(NC_CHUNKS):
                nc.sync.dma_start(wv_sb[:, c * 512:(c + 1) * 512],
                                  wv[c * 128:(c + 1) * 128, :])
            stage_vq(0)
            stage_vq(1)

            # ---- filler machinery: deadline-ordered group thunks ----
            def run_fills(fills, kc):
                for f in fills.pop(kc, []):
                    f()

            def add_fill(fills, kc, f):
                fills.setdefault(kc, []).append(f)

            def kproj_at(fills, kc0, p, kb):
                add_fill(fills, kc0,
                         lambda p=p, kb=kb: proj_group_run(
                             "k", kstg[kb], wk_sb, khTb[p][kb], p,
                             f"k{p}{kb}"))

            def qproj_at(fills, kc0, p, qb):
                add_fill(fills, kc0,
                         lambda p=p, qb=qb: proj_group_run(
                             "q", qstg[qb], wq_sb, qhTb[p][qb], p,
                             f"q{p}{qb}"))

            wo_queue = []

            def vproj_at(fills, kc0, t, half):
                add_fill(fills, kc0,
                         lambda t=t, half=half: vproj_group_run(t, half))

            def build_fills(blk, qb):
                """blk1's k/v/q prerequisites are spread over blk0's
                under-loaded qb1-3 windows so no single window carries
                more projection work than the exp stream's slack."""
                fills = {}
                if qb == 0:
                    if blk == 0:
                        for kb in range(1, NQB):
                            def stage_kb(kb=kb):
                                kstg[kb] = stage_block(kst_pool, kT, kb)
                            add_fill(fills, max(0, 4 * kb - 8), stage_kb)

                        def stage_wo():
                            for p in range(NP):
                                nc.sync.dma_start(
                                    wo_sb[:, p * 1024:(p + 1) * 1024],
                                    wo[p * 128:(p + 1) * 128, :])
                        add_fill(fills, 12, stage_wo)
                        for t in range(NKC):
                            vproj_at(fills, t + 2, t, 0)
                        add_fill(fills, 4, lambda: stage_vq(2))
                        add_fill(fills, 8, lambda: stage_vq(3))
                        for kb in range(1, NQB):
                            kproj_at(fills, 4 * kb - 2, 0, kb)
                            kproj_at(fills, 4 * kb - 1, 1, kb)
                    else:
                        # blk1-qb0: k blocks 2,3 for pairs 2,3 and the
                        # second v-half chunks 8-15 (0-7 + k blocks
                        # 0,1 were pre-built during blk0)
                        add_fill(fills, 4, lambda: stage_vq(2))
                        add_fill(fills, 8, lambda: stage_vq(3))
                        for t in range(8, NKC):
                            vproj_at(fills, t + 2, t, 1)
                        kproj_at(fills, 5, 2, 2)
                        kproj_at(fills, 6, 3, 2)
                        kproj_at(fills, 9, 2, 3)
                        kproj_at(fills, 10, 3, 3)
                else:
                    # steady state: consume queued Wo work
                    nwo = min(8, len(wo_queue))
                    for i in range(nwo):
                        add_fill(fills, 1 + (i * 14) // max(nwo, 1),
                                 wo_queue.pop(0))
                if blk == 0:
                    if qb == 1:
                        kproj_at(fills, 4, 2, 0)
                        kproj_at(fills, 10, 3, 0)
                    elif qb == 2:
                        kproj_at(fills, 2, 2, 1)
                        kproj_at(fills, 8, 3, 1)
                        add_fill(fills, 0, lambda: stage_vq(0))
                        add_fill(fills, 8, lambda: stage_vq(1))
                        vproj_at(fills, 4, 0, 1)
                        vproj_at(fills, 6, 1, 1)
                        vproj_at(fills, 10, 2, 1)
                        vproj_at(fills, 12, 3, 1)
                    elif qb == 3:
                        vproj_at(fills, 1, 4, 1)
                        vproj_at(fills, 4, 5, 1)
                        vproj_at(fills, 7, 6, 1)
                        vproj_at(fills, 10, 7, 1)
                # stage/project next qb's q for this block's pairs
                pA, pB = 2 * blk, 2 * blk + 1
                if qb + 1 < NQB:
                    nqb = qb + 1

                    def stage_qn(nqb=nqb):
                        qstg[nqb] = stage_block(qst_pool, qT, nqb)
                    add_fill(fills, 6, stage_qn)
                    qproj_at(fills, 8, pA, nqb)
                    qproj_at(fills, 12, pB, nqb)
                elif blk == 0:
                    # blk1-qb0's q block 0 for pairs 2,3
                    def stage_q0():
                        qstg[0] = stage_block(qst_pool, qT, 0)
                    add_fill(fills, 6, stage_q0)
                    qproj_at(fills, 8, 2, 0)
                    qproj_at(fills, 12, 3, 0)
                return fills

            # ---- main loop ----
            ntbl = [[None] * NP for _ in range(NQB)]
            for blk in range(2):
                pA, pB = 2 * blk, 2 * blk + 1
                for qb in range(NQB):
                    fills = build_fills(blk, qb)
                    mixA = mix_pool.tile([128, 512], F32, tag="mix",
                                         name=f"mixA{blk}{qb}")
                    mixB = mix_pool.tile([128, 512], F32, tag="mix",
                                         name=f"mixB{blk}{qb}")
                    sums = sums_pool.tile([128, 512], F32, tag="sums",
                                          name=f"sums{blk}{qb}")
                    nc.vector.memset(mixA[:], 0.0)
                    nc.vector.memset(mixB[:], 0.0)
                    nc.vector.memset(sums[:], 0.0)
                    pend = []
                    for kc in range(NKC):
                        run_fills(fills, kc)
                        scA = sc_pool.tile([128, 1024], F32, tag="sc")
                        scores_wave(pA, qb, kc, scA)
                        scB = sc_pool.tile([128, 1024], F32, tag="sc")
                        scores_wave(pB, qb, kc, scB)
                        exA = exp_pool.tile([128, 1024], BF16, tag="exp")
                        nc.scalar.activation(exA[:], scA[:], EXP)
                        exB = exp_pool.tile([128, 1024], BF16, tag="exp")
                        nc.scalar.activation(exB[:], scB[:], EXP)
                        # mix/sums lag 2 kc behind exp so neither a slow
                        # v-projection nor the ACT dependency can stall
                        # the scores pipeline
                        if len(pend) == 2:
                            eA, eB = pend.pop(0)
                            mix_wave(pA, kc - 2, eA, mixA)
                            mix_wave(pB, kc - 2, eB, mixB)
                            sums_wave(kc - 2, eA, eB, sums)
                        pend.append((exA, exB))
                    # drain leftover fillers, then the last waves
                    for k in sorted(fills.keys()):
                        run_fills(fills, k)
                    for i, (eA, eB) in enumerate(pend):
                        mix_wave(pA, NKC - 2 + i, eA, mixA)
                        mix_wave(pB, NKC - 2 + i, eB, mixB)
                        sums_wave(NKC - 2 + i, eA, eB, sums)
                    rec = recip_sums(sums, f"{blk}_{qb}")
                    ntbl[qb][pA] = normalize(mixA, rec, 0, f"{pA}_{qb}")
                    ntbl[qb][pB] = normalize(mixB, rec, 2, f"{pB}_{qb}")
                    if blk == 1:
                        if qb < NQB - 1:
                            for tt in range(4):
                                for dh in range(2):
                                    wo_queue.append(
                                        lambda qb=qb, tt=tt, dh=dh:
                                        wo_group_run(qb, ntbl[qb], tt, dh))
                        else:
                            # tail: the attention psum banks are idle
                            # now - run all 8 Wo groups concurrently,
                            # p-major, one accumulator bank each
                            slots = []
                            for i in range(2):
                                sct = sc_pool.tile([128, 1024], F32,
                                                   tag="sc", name=f"fw{i}")
                                slots += [sct[:, 0:512], sct[:, 512:1024]]
                            for i in range(2):
                                mt = mix_pool.tile([128, 512], F32,
                                                   tag="mix", name=f"fwm{i}")
                                slots.append(mt[:])
                            st = sums_pool.tile([128, 512], F32,
                                                tag="sums", name="fws")
                            slots.append(st[:])
                            at = aux_pool.tile([128, 512], F32,
                                               tag="aux", name="fwa")
                            slots.append(at[:])
                            combos = [(tt, dh) for tt in range(4)
                                      for dh in range(2)]
                            nts = ntbl[qb]
                            for p in range(NP):
                                for i, (tt, dh) in enumerate(combos):
                                    nc.tensor.matmul(
                                        slots[i],
                                        lhsT=nts[p][:, tt * 128:
                                                    (tt + 1) * 128],
                                        rhs=wo_sb[:, p * 1024 + dh * 512:
                                                  p * 1024 +
                                                  (dh + 1) * 512],
                                        start=(p == 0), stop=(p == NP - 1))
                            for i, (tt, dh) in enumerate(combos):
                                osb = osb_pool.tile([128, 512], BF16,
                                                    tag="osb")
                                nc.vector.tensor_copy(osb[:], slots[i])
                                nc.sync.dma_start(
                                    out[qb * 512 + tt * 128:
                                        qb * 512 + (tt + 1) * 128,
                                        dh * 512:(dh + 1) * 512], osb[:])
            for f in wo_queue:
                f()

    nc.compile()
    return nc


def _get_nc():
    if "nc" not in _COMPILED:
        _COMPILED["nc"] = _build_nc()
    return _COMPILED["nc"]


def _shard_inputs(q, k, v, mask, Wq, Wk, Wv, Wo):
    """Build the per-core input maps (host-side layout prep)."""
    import ml_dtypes

    bf16 = ml_dtypes.bfloat16
    in_maps = []
    maskf = np.asarray(mask).astype(np.float32)
    q = np.asarray(q, np.float32)
    k = np.asarray(k, np.float32)
    v = np.asarray(v, np.float32)
    Wq = np.asarray(Wq, np.float32)
    Wk = np.asarray(Wk, np.float32)
    Wv = np.asarray(Wv, np.float32)
    Wo = np.asarray(Wo, np.float32)
    scale = np.float32(1.0 / np.sqrt(DK))
    for c in range(NCORES):
        b, hg = c // 2, c % 2
        hs = hg * HC
        m = {
            "qT": np.ascontiguousarray(q[b].T).astype(bf16),
            "kT": np.ascontiguousarray(k[b].T).astype(bf16),
            "vT": np.ascontiguousarray(v[b].T).astype(bf16),
            # head-major col blocks; fold 1/sqrt(dk) into Wq
            "wq": np.ascontiguousarray(
                Wq[hs:hs + HC].transpose(1, 0, 2).reshape(D, HC * DK) * scale
            ).astype(bf16),
            "wk": np.ascontiguousarray(
                Wk[hs:hs + HC].transpose(1, 0, 2).reshape(D, HC * DK)
            ).astype(bf16),
            "wv": np.ascontiguousarray(
                Wv[hs:hs + HC].transpose(1, 0, 2).reshape(D, HC * DV)
            ).astype(bf16),
            "wo": np.ascontiguousarray(Wo[hs * DV:(hs + HC) * DV]).astype(bf16),
            "maskr": np.ascontiguousarray(
                maskf[b].reshape(NKC, 128).T).astype(np.float32),
        }
        in_maps.append(m)
    return in_maps


def kernel(q, k, v, mask, Wq, Wk, Wv, Wo, _trace=False):
    from concourse.bass_utils import run_bass_kernel_spmd

    nc = _get_nc()
    in_maps = _shard_inputs(q, k, v, mask, Wq, Wk, Wv, Wo)
    res = run_bass_kernel_spmd(nc, in_maps, list(range(NCORES)),
                               trace=_trace)
    out = np.zeros((B, S, D), np.float32)
    for c in range(NCORES):
        out[c // 2] += np.asarray(res.results[c]["out"], np.float32)
    if _trace:
        _COMPILED["last_result"] = res
    return out
